# revision 45
# baseline (speedup 1.0000x reference)
"""Trainium2 Bass kernel for nn_JinaPairTraining (dense CE + late-interaction
maxsim CE + KL between the two softmax distributions).

Sharding: data-parallel over the query batch dim Bq. Rows are assigned to the
8 cores to balance valid-q-token counts; every core receives the full
(mask-packed) pos side and computes its rows of the raw maxsim matrix
S_raw[row, doc] = sum_{valid q} max_{valid p} sim.  The host does everything
else: the dense [32,32] logits (tiny), the row softmax / CE / KL in float64,
and the final mean.  Only the O(B^2 T^2 D) sim work runs on device.

Mask packing (exact, no approximation):
  * q side: only valid q tokens are shipped, packed into chunks of 128
    (crossing row boundaries).  The masked one-hot stationary (qoh) of the
    final sum-over-q matmul routes each token slot to its row; pad slots get
    weight 0.
  * p side: only valid pos tokens are shipped.  Tokens are pair-folded
    (max(s0, s1) = s1 + relu(s0 - s1), computed as PE matmuls + one ACT relu
    + an identity-matmul accumulate).  Docs are sorted by pair count and
    grouped into 4 regions of 8 docs; each region pads its docs to the
    region max with duplicate pairs (duplicates never change a max).
  * the kernel is compiled per (chunk-count, region-widths) signature and
    cached; all-ones masks degenerate to the dense full-size layout.
"""

import os
import sys

import numpy as np

for _p in ("/opt/trn_rl_repo",):
    if _p not in sys.path and os.path.isdir(_p):
        sys.path.insert(0, _p)

import concourse.bacc as bacc
import concourse.tile as tile
from concourse import mybir
from concourse.bass_utils import run_bass_kernel_spmd

B, T, D = 32, 256, 128
TAU = 0.02
EPS = 1e-8
NCORES = 8
BPC = B // NCORES  # 4 query rows per core
NREG = 4           # pos regions (8 docs each, sorted by valid-pair count)
DPR = B // NREG    # docs per region

F32 = mybir.dt.float32
BF16 = mybir.dt.bfloat16
AX = mybir.AxisListType
ACT = mybir.ActivationFunctionType


def _build_kernel(nj, groups):
    """nj: q chunks per core; groups: per-region (n_docs, pairs-per-doc)."""
    nc = bacc.Bacc(None, target_bir_lowering=False, debug=False)

    nreg = len(groups)
    totw = sum(n * s * g for n, s, g in groups)
    pT_d = nc.dram_tensor("pT", [D, totw], BF16, kind="ExternalInput")
    qT_d = nc.dram_tensor("qT", [D, nj * 128], BF16, kind="ExternalInput")
    ident_d = nc.dram_tensor("identity", [128, 128], BF16, kind="ExternalInput")
    # Doc-major slab for all regions but the last; the last region is
    # j-major so its per-chunk slices stay DMA-contiguous.
    nlast = groups[-1][0]
    out_d = nc.dram_tensor("out", [128, B - nlast, nj], F32, kind="ExternalOutput")
    outl_d = nc.dram_tensor("outl", [128, nj, nlast], F32, kind="ExternalOutput")

    roff = np.cumsum([0] + [n * s * g for n, s, g in groups]).tolist()
    doff = np.cumsum([0] + [n for n, _, _ in groups]).tolist()

    with tile.TileContext(nc) as tc:
        with tc.tile_pool(name="sb", bufs=1) as sb:
            # PE warm-up: dummy matmuls during the input-DMA shadow so the
            # p-state ramp (HAM) is done before the first real matmul.
            with tc.tile_pool(name="warm", bufs=1, space="PSUM") as wp:
                wsrc = sb.tile([128, 512], BF16)
                nc.vector.memset(wsrc, 0.0)
                wdst = wp.tile([128, 512], F32)
                for _ in range(6):
                    nc.tensor.matmul(wdst, wsrc[:, :128], wsrc, start=True, stop=True)
            # qT + smalls ride the ACT queue; p regions stream on the SP
            # queue in parallel.
            qT = sb.tile([D, nj * 128], BF16)
            nc.scalar.dma_start(out=qT, in_=qT_d[:, :])
            ident = sb.tile([128, 128], BF16)
            nc.sync.dma_start(out=ident, in_=ident_d[:, :])
            pT = sb.tile([D, 2 * totw], BF16)
            for r in range(nreg):
                eng = nc.sync if r % 2 == 0 else nc.scalar
                eng.dma_start(
                    out=pT[:, roff[r] : roff[r + 1]],
                    in_=pT_d[:, roff[r] : roff[r + 1]],
                )

            # mx[q, c, j]: per q chunk j, per pos doc c (sorted order), the
            # masked max over that doc's tokens.  Doc-major layout keeps each
            # region's slab contiguous so it can be DMAed out as soon as the
            # region finishes; the host does the masked sum over q.  The last
            # region is j-major (mxl) for per-chunk outgoing DMAs.
            mx = sb.tile([128, B - nlast, nj], F32)
            mxl = sb.tile([128, nj, nlast], F32)

            with (
                tc.tile_pool(name="pb", bufs=2, space="PSUM") as pb,
                tc.tile_pool(name="rp", bufs=3) as rp,
            ):
                for r, (nd, s, G) in enumerate(groups):
                    w = nd * s
                    blk = [
                        pT[:, roff[r] + t * w : roff[r] + (t + 1) * w]
                        for t in range(G)
                    ]
                    nsplit = 2 if w > 512 else 1
                    hw_ = w // nsplit
                    gpr = nd // nsplit
                    for j in range(nj):
                        qj = qT[:, j * 128 : (j + 1) * 128]
                        # Chain: T_t = Q@(grp_t - grp_{t+1}) + I@relu(T_{t-1})
                        # so T_t holds max(grp_1..grp_t) - grp_{t+1} ... the
                        # final M = Q@grp_G + I@relu(T_{G-1}) is the running
                        # columnwise max over all G groups.
                        relu_prev = None
                        for t in range(G - 1):
                            ps_t = pb.tile([128, w], F32, name="ps_t")
                            for k in range(0, w, 512):
                                sl = slice(k, min(k + 512, w))
                                nc.tensor.matmul(
                                    ps_t[:, sl],
                                    qj,
                                    blk[t][:, sl],
                                    start=True,
                                    stop=(relu_prev is None),
                                )
                            if relu_prev is not None:
                                for k in range(0, w, 512):
                                    sl = slice(k, min(k + 512, w))
                                    nc.tensor.matmul(
                                        ps_t[:, sl],
                                        ident,
                                        relu_prev[:, sl],
                                        start=False,
                                        stop=True,
                                    )
                            relu_sb = rp.tile([128, w], BF16, name=f"relu{t % 2}")
                            nc.scalar.activation(relu_sb, ps_t, ACT.Relu)
                            relu_prev = relu_sb
                        # Final stage: M as two half tiles (each <= 1 PSUM
                        # bank) when wide, so each is freed right after its
                        # own (short) reduce.
                        ps_ms = [
                            pb.tile([128, hw_], F32, name=f"ps_m{h}")
                            for h in range(nsplit)
                        ]
                        for h in range(nsplit):
                            for k in range(0, hw_, 512):
                                sl = slice(k, min(k + 512, hw_))
                                nc.tensor.matmul(
                                    ps_ms[h][:, sl],
                                    qj,
                                    blk[G - 1][:, h * hw_ + sl.start : h * hw_ + sl.stop],
                                    start=True,
                                    stop=(relu_prev is None),
                                )
                        if relu_prev is not None:
                            for h in range(nsplit):
                                for k in range(0, hw_, 512):
                                    sl = slice(k, min(k + 512, hw_))
                                    nc.tensor.matmul(
                                        ps_ms[h][:, sl],
                                        ident,
                                        relu_prev[
                                            :, h * hw_ + sl.start : h * hw_ + sl.stop
                                        ],
                                        start=False,
                                        stop=True,
                                    )
                        for h in range(nsplit):
                            if r < nreg - 1:
                                mout = mx[
                                    :, doff[r] + h * gpr : doff[r] + (h + 1) * gpr, j
                                ]
                            else:
                                mout = mxl[:, j, h * gpr : (h + 1) * gpr]
                            nc.vector.reduce_max(
                                out=mout,
                                in_=ps_ms[h].rearrange("p (g s) -> p g s", s=s),
                                axis=AX.X,
                            )
                    # Ship this region's mx slab while later regions compute.
                    # The last region goes out per-chunk so the final DMA is
                    # tiny and starts right at the last reduce.
                    if r < nreg - 1:
                        nc.sync.dma_start(
                            out=out_d[:, doff[r] : doff[r + 1], :],
                            in_=mx[:, doff[r] : doff[r + 1], :],
                        )
                    else:
                        for jj in range(nj):
                            nc.sync.dma_start(
                                out=outl_d[:, jj], in_=mxl[:, jj]
                            )

    nc.compile()
    return nc


_NC_CACHE = {}
_LAST_NC = None


def _get_nc(nj=None, widths=None):
    global _LAST_NC
    if nj is None:
        return _LAST_NC
    key = (nj, tuple(widths))
    if key not in _NC_CACHE:
        _NC_CACHE[key] = _build_kernel(nj, widths)
    _LAST_NC = _NC_CACHE[key]
    return _LAST_NC


def _pad4(x):
    return (x + 3) & ~3


def _plan(q_mask, p_mask):
    """Row->core assignment, q chunk count, pos doc order + region widths."""
    qlen = q_mask.sum(axis=1).astype(int)
    # Balance valid-q counts across cores (4 rows each): greedy LPT, then
    # pairwise-swap refinement to minimize the max core sum (which sets the
    # compiled chunk count for every core).
    order = np.argsort(-qlen, kind="stable")
    sums = [0] * NCORES
    counts = [0] * NCORES
    rows_per_core = [[] for _ in range(NCORES)]
    for b in order:
        cands = [c for c in range(NCORES) if counts[c] < BPC]
        c = min(cands, key=lambda c: sums[c])
        rows_per_core[c].append(int(b))
        sums[c] += int(qlen[b])
        counts[c] += 1
    improved = True
    while improved:
        improved = False
        hi = int(np.argmax(sums))
        for lo in sorted(range(NCORES), key=lambda c: sums[c]):
            if lo == hi:
                continue
            for i, bh in enumerate(rows_per_core[hi]):
                for k, bl in enumerate(rows_per_core[lo]):
                    delta = int(qlen[bh]) - int(qlen[bl])
                    if delta <= 0:
                        continue
                    new_hi = sums[hi] - delta
                    new_lo = sums[lo] + delta
                    if max(new_hi, new_lo) < sums[hi]:
                        rows_per_core[hi][i], rows_per_core[lo][k] = bl, bh
                        sums[hi], sums[lo] = new_hi, new_lo
                        improved = True
                        break
                if improved:
                    break
            if improved:
                break
    nj = max(1, (max(sums) + 127) // 128)

    # Pos docs sorted by valid-pair count, then DP-partitioned into regions
    # (even doc counts, width <= 1024) minimizing the predicted per-unit
    # bottleneck-engine time.
    plen = p_mask.sum(axis=1).astype(int)
    pairs = (plen + 1) // 2
    doc_order = np.argsort(pairs, kind="stable")
    sp = [int(max(1, pairs[doc_order[i]])) for i in range(B)]

    # Fixed even regions measured faster than DP-partitioned variants: the
    # ascending-width groups keep the first DMA small and the pipeline
    # uniform.  Per region, choose the fold-chain depth G (max over G groups
    # via G-1 relu stages) to balance the ACT (relu), DVE (reduce) and PE
    # (matmul) totals.
    lmax = [int(max(1, plen[doc_order[(r + 1) * DPR - 1]])) for r in range(NREG)]

    def costs(L, G):
        s = max(1, -(-L // G))
        w = DPR * s
        pe = (2 * G - 1) * w * 0.4167 + (2 * G + 1) * 30
        act = (G - 1) * (0.833 * w + 185)
        dve = 1.0417 * w + 130 * (1 if w <= 512 else 2)
        return w, pe, act, dve

    best = None
    for g0 in (1, 2, 3, 4):
        for g1 in (1, 2, 3, 4):
            for g2 in (2, 3, 4):
                for g3 in (2, 3, 4):
                    gs = (g0, g1, g2, g3)
                    tot = [0.0, 0.0, 0.0]
                    ok = True
                    for r, G in enumerate(gs):
                        w, pe, act, dve = costs(lmax[r], G)
                        if w > 1024:
                            ok = False
                            break
                        tot[0] += pe
                        tot[1] += act
                        tot[2] += dve
                    if not ok:
                        continue
                    score = max(tot) + 0.02 * sum(tot)
                    if best is None or score < best[0]:
                        best = (score, gs)
    gs = best[1]
    groups = [
        (DPR, max(1, -(-lmax[r] // gs[r])), gs[r]) for r in range(NREG)
    ]
    return rows_per_core, nj, doc_order, groups


def _prep_pos(pm, pmask, doc_order, groups):
    """Packed [D, sum_r G_r*w_r] bf16 pos tensor.

    Per region (chain depth G): blocks [grp_1-grp_2 | grp_2-grp_3 | ... |
    grp_{G-1}-grp_G | grp_G], each [w_r, D] transposed.  Each doc's valid
    tokens are distributed over G groups of s slots, padded with duplicates
    of token 0 (duplicates never change a max).
    """
    import ml_dtypes

    blocks = []
    d0 = 0
    for nd, s, G in groups:
        grps = [np.zeros((nd * s, D), np.float32) for _ in range(G)]
        for i, c in enumerate(doc_order[d0 : d0 + nd]):
            tok = pm[c][pmask[c]]  # [L, D] valid tokens
            L = len(tok)
            for t in range(G):
                seg = tok[t * s : (t + 1) * s]
                if len(seg) < s:
                    pad = np.repeat(tok[0:1], s - len(seg), axis=0)
                    seg = np.concatenate([seg, pad], axis=0) if len(seg) else pad
                grps[t][i * s : (i + 1) * s] = seg
        for t in range(G - 1):
            blocks.append((grps[t] - grps[t + 1]).T)
        blocks.append(grps[G - 1].T)
        d0 += nd
    pT = np.ascontiguousarray(np.concatenate(blocks, axis=1)).astype(
        ml_dtypes.bfloat16
    )
    return pT


def _prep_in_maps(query_multi, pos_multi, q_mask, p_mask, plan):
    import ml_dtypes

    rows_per_core, nj, doc_order, groups = plan
    qm = np.ascontiguousarray(np.asarray(query_multi, np.float32))
    pm = np.ascontiguousarray(np.asarray(pos_multi, np.float32))
    qmask = np.asarray(q_mask).astype(bool)
    pmask = np.asarray(p_mask).astype(bool)

    pT = _prep_pos(pm, pmask, doc_order, groups)
    ident = np.eye(128, dtype=ml_dtypes.bfloat16)

    in_maps = []
    qohs = []
    for c in range(NCORES):
        qtok = np.zeros((nj * 128, D), np.float32)
        qoh = np.zeros((nj * 128, BPC), np.float32)
        pos = 0
        for i, b in enumerate(rows_per_core[c]):
            tok = qm[b][qmask[b]]
            n = len(tok)
            qtok[pos : pos + n] = tok
            qoh[pos : pos + n, i] = 1.0
            pos += n
        qT = np.ascontiguousarray(qtok.T).astype(ml_dtypes.bfloat16)
        in_maps.append({"pT": pT, "qT": qT, "identity": ident})
        qohs.append(qoh)  # [nj*128, BPC] host-side sum weights
    return in_maps, qohs


def _host_losses(dense_sim, S_late):
    """Float64 replica of the reference softmax/CE/KL tail."""

    def softmax_and_logp(z):
        m = z.max(axis=1, keepdims=True)
        e = np.exp(z - m)
        den = e.sum(axis=1, keepdims=True)
        return e / den, (z - m) - np.log(den)

    zd = dense_sim / TAU
    zl = S_late / TAU
    dp, logp_d = softmax_and_logp(zd)
    lp, logp_l = softmax_and_logp(zl)
    idx = np.arange(B)
    single = -logp_d[idx, idx].mean()
    multi = -logp_l[idx, idx].mean()
    kl = (dp * np.log((dp + EPS) / (lp + EPS))).sum(axis=1).mean()
    return single, multi, kl


def run(inputs: dict, trace: bool = False):
    """Run the spmd kernel; returns (loss tuple, BassKernelResults)."""
    qmask = np.asarray(inputs["q_mask"]).astype(bool)
    pmask = np.asarray(inputs["p_mask"]).astype(bool)
    plan = _plan(qmask, pmask)
    rows_per_core, nj, doc_order, groups = plan

    nc = _get_nc(nj, groups)
    in_maps, qohs = _prep_in_maps(
        inputs["query_multi"], inputs["pos_multi"], qmask, pmask, plan
    )
    res = run_bass_kernel_spmd(nc, in_maps, core_ids=list(range(NCORES)), trace=trace)

    # Assemble S_raw in original (row, doc) order.  Device output is
    # mx[slot, doc, chunk] (+ j-major slab for the last region); the masked
    # sum over q slots is a tiny host einsum.
    nlast = groups[-1][0]
    S_raw = np.zeros((B, B), np.float64)
    for c in range(NCORES):
        mx = np.asarray(res.results[c]["out"], np.float64)  # [128, B-nlast, nj]
        mxl = np.asarray(res.results[c]["outl"], np.float64)  # [128, nj, nlast]
        m_a = mx.transpose(2, 0, 1).reshape(nj * 128, B - nlast)
        m_b = mxl.transpose(1, 0, 2).reshape(nj * 128, nlast)
        mx2 = np.concatenate([m_a, m_b], axis=1)  # [slot, sorted doc]
        block = qohs[c].T @ mx2  # [BPC, B]
        for i, b in enumerate(rows_per_core[c]):
            S_raw[b, doc_order] = block[i]

    t_i = np.maximum(qmask.sum(axis=1), 1).astype(np.float64)
    S_late = S_raw / t_i[:, None]

    qs = np.asarray(inputs["query_single"], np.float64)
    ps = np.asarray(inputs["pos_single"], np.float64)
    dense_sim = qs @ ps.T

    single, multi, kl = _host_losses(dense_sim, S_late)
    total = single + multi + kl
    out = (np.float32(total), np.float32(single), np.float32(multi), np.float32(kl))
    return out, res


def kernel(query_single, pos_single, query_multi, pos_multi, q_mask, p_mask):
    out, _ = run(
        {
            "query_single": query_single,
            "pos_single": pos_single,
            "query_multi": query_multi,
            "pos_multi": pos_multi,
            "q_mask": q_mask,
            "p_mask": p_mask,
        }
    )
    return out


# revision 46
# speedup vs baseline: 1.1269x; 1.1269x over previous
"""Trainium2 Bass kernel for nn_JinaPairTraining (dense CE + late-interaction
maxsim CE + KL between the two softmax distributions).

Sharding: data-parallel over the query batch dim Bq. Rows are assigned to the
8 cores to balance valid-q-token counts; every core receives the full
(mask-packed) pos side and computes its rows of the raw maxsim matrix
S_raw[row, doc] = sum_{valid q} max_{valid p} sim.  The host does everything
else: the dense [32,32] logits (tiny), the row softmax / CE / KL in float64,
and the final mean.  Only the O(B^2 T^2 D) sim work runs on device.

Mask packing (exact, no approximation):
  * q side: only valid q tokens are shipped, packed into chunks of 128
    (crossing row boundaries).  The masked one-hot stationary (qoh) of the
    final sum-over-q matmul routes each token slot to its row; pad slots get
    weight 0.
  * p side: only valid pos tokens are shipped.  Tokens are pair-folded
    (max(s0, s1) = s1 + relu(s0 - s1), computed as PE matmuls + one ACT relu
    + an identity-matmul accumulate).  Docs are sorted by pair count and
    grouped into 4 regions of 8 docs; each region pads its docs to the
    region max with duplicate pairs (duplicates never change a max).
  * the kernel is compiled per (chunk-count, region-widths) signature and
    cached; all-ones masks degenerate to the dense full-size layout.
"""

import os
import sys

import numpy as np

for _p in ("/opt/trn_rl_repo",):
    if _p not in sys.path and os.path.isdir(_p):
        sys.path.insert(0, _p)

import concourse.bacc as bacc
import concourse.tile as tile
from concourse import mybir
from concourse.bass_utils import run_bass_kernel_spmd

B, T, D = 32, 256, 128
TAU = 0.02
EPS = 1e-8
NCORES = 8
BPC = B // NCORES  # 4 query rows per core
NREG = 4           # pos regions (8 docs each, sorted by valid-pair count)
DPR = B // NREG    # docs per region

F32 = mybir.dt.float32
BF16 = mybir.dt.bfloat16
AX = mybir.AxisListType
ACT = mybir.ActivationFunctionType


def _build_kernel(nj, groups):
    """nj: q chunks per core; groups: per-region (n_docs, pairs-per-doc)."""
    nc = bacc.Bacc(None, target_bir_lowering=False, debug=False)

    nreg = len(groups)
    totw = sum(n * s * g for n, s, g in groups)
    pT_d = nc.dram_tensor("pT", [D, totw], BF16, kind="ExternalInput")
    qT_d = nc.dram_tensor("qT", [D, nj * 128], BF16, kind="ExternalInput")
    ident_d = nc.dram_tensor("identity", [128, 128], BF16, kind="ExternalInput")
    # Doc-major slab for all regions but the last; the last region is
    # j-major so its per-chunk slices stay DMA-contiguous.
    nlast = groups[-1][0]
    out_d = nc.dram_tensor("out", [128, B - nlast, nj], F32, kind="ExternalOutput")
    outl_d = nc.dram_tensor("outl", [128, nj, nlast], F32, kind="ExternalOutput")

    roff = np.cumsum([0] + [n * s * g for n, s, g in groups]).tolist()
    doff = np.cumsum([0] + [n for n, _, _ in groups]).tolist()

    with tile.TileContext(nc) as tc:
        with tc.tile_pool(name="sb", bufs=1) as sb:
            # PE warm-up: dummy matmuls during the input-DMA shadow so the
            # p-state ramp (HAM) is done before the first real matmul.
            with tc.tile_pool(name="warm", bufs=1, space="PSUM") as wp:
                wsrc = sb.tile([128, 512], BF16)
                nc.vector.memset(wsrc, 0.0)
                wdst = wp.tile([128, 512], F32)
                for _ in range(6):
                    nc.tensor.matmul(wdst, wsrc[:, :128], wsrc, start=True, stop=True)
            # qT + smalls ride the ACT queue; p regions stream on the SP
            # queue in parallel.
            qT = sb.tile([D, nj * 128], BF16)
            nc.scalar.dma_start(out=qT, in_=qT_d[:, :])
            ident = sb.tile([128, 128], BF16)
            nc.sync.dma_start(out=ident, in_=ident_d[:, :])
            pT = sb.tile([D, 2 * totw], BF16)
            for r in range(nreg):
                eng = nc.sync if r % 2 == 0 else nc.scalar
                eng.dma_start(
                    out=pT[:, roff[r] : roff[r + 1]],
                    in_=pT_d[:, roff[r] : roff[r + 1]],
                )

            # mx[q, c, j]: per q chunk j, per pos doc c (sorted order), the
            # masked max over that doc's tokens.  Doc-major layout keeps each
            # region's slab contiguous so it can be DMAed out as soon as the
            # region finishes; the host does the masked sum over q.  The last
            # region is j-major (mxl) for per-chunk outgoing DMAs.
            mx = sb.tile([128, B - nlast, nj], F32)
            mxl = sb.tile([128, nj, nlast], F32)

            with (
                tc.tile_pool(name="pb", bufs=2, space="PSUM") as pb,
                tc.tile_pool(name="rp", bufs=3) as rp,
            ):
                for r, (nd, s, G) in enumerate(groups):
                    w = nd * s
                    blk = [
                        pT[:, roff[r] + t * w : roff[r] + (t + 1) * w]
                        for t in range(G)
                    ]
                    nsplit = 2 if w > 512 else 1
                    hw_ = w // nsplit
                    gpr = nd // nsplit
                    for j in range(nj):
                        qj = qT[:, j * 128 : (j + 1) * 128]
                        # Chain: T_t = Q@(grp_t - grp_{t+1}) + I@relu(T_{t-1})
                        # so T_t holds max(grp_1..grp_t) - grp_{t+1} ... the
                        # final M = Q@grp_G + I@relu(T_{G-1}) is the running
                        # columnwise max over all G groups.
                        relu_prev = None
                        for t in range(G - 1):
                            ps_t = pb.tile([128, w], F32, name="ps_t")
                            for k in range(0, w, 512):
                                sl = slice(k, min(k + 512, w))
                                nc.tensor.matmul(
                                    ps_t[:, sl],
                                    qj,
                                    blk[t][:, sl],
                                    start=True,
                                    stop=(relu_prev is None),
                                )
                            if relu_prev is not None:
                                for k in range(0, w, 512):
                                    sl = slice(k, min(k + 512, w))
                                    nc.tensor.matmul(
                                        ps_t[:, sl],
                                        ident,
                                        relu_prev[:, sl],
                                        start=False,
                                        stop=True,
                                    )
                            relu_sb = rp.tile([128, w], BF16, name=f"relu{t % 2}")
                            nc.scalar.activation(relu_sb, ps_t, ACT.Relu)
                            relu_prev = relu_sb
                        # Final stage: M as two half tiles (each <= 1 PSUM
                        # bank) when wide, so each is freed right after its
                        # own (short) reduce.
                        ps_ms = [
                            pb.tile([128, hw_], F32, name=f"ps_m{h}")
                            for h in range(nsplit)
                        ]
                        for h in range(nsplit):
                            for k in range(0, hw_, 512):
                                sl = slice(k, min(k + 512, hw_))
                                nc.tensor.matmul(
                                    ps_ms[h][:, sl],
                                    qj,
                                    blk[G - 1][:, h * hw_ + sl.start : h * hw_ + sl.stop],
                                    start=True,
                                    stop=(relu_prev is None),
                                )
                        if relu_prev is not None:
                            for h in range(nsplit):
                                for k in range(0, hw_, 512):
                                    sl = slice(k, min(k + 512, hw_))
                                    nc.tensor.matmul(
                                        ps_ms[h][:, sl],
                                        ident,
                                        relu_prev[
                                            :, h * hw_ + sl.start : h * hw_ + sl.stop
                                        ],
                                        start=False,
                                        stop=True,
                                    )
                        for h in range(nsplit):
                            if r < nreg - 1:
                                mout = mx[
                                    :, doff[r] + h * gpr : doff[r] + (h + 1) * gpr, j
                                ]
                            else:
                                mout = mxl[:, j, h * gpr : (h + 1) * gpr]
                            nc.vector.reduce_max(
                                out=mout,
                                in_=ps_ms[h].rearrange("p (g s) -> p g s", s=s),
                                axis=AX.X,
                            )
                    # Ship this region's mx slab while later regions compute.
                    # The last region goes out per-chunk so the final DMA is
                    # tiny and starts right at the last reduce.
                    if r < nreg - 1:
                        nc.sync.dma_start(
                            out=out_d[:, doff[r] : doff[r + 1], :],
                            in_=mx[:, doff[r] : doff[r + 1], :],
                        )
                    else:
                        for jj in range(nj):
                            nc.sync.dma_start(
                                out=outl_d[:, jj], in_=mxl[:, jj]
                            )

    nc.compile()
    return nc


_NC_CACHE = {}
_LAST_NC = None


def _get_nc(nj=None, widths=None):
    global _LAST_NC
    if nj is None:
        return _LAST_NC
    key = (nj, tuple(widths))
    if key not in _NC_CACHE:
        _NC_CACHE[key] = _build_kernel(nj, widths)
    _LAST_NC = _NC_CACHE[key]
    return _LAST_NC


def _pad4(x):
    return (x + 3) & ~3


def _plan(q_mask, p_mask):
    """Row->core assignment, q chunk count, pos doc order + region widths."""
    qlen = q_mask.sum(axis=1).astype(int)
    # Balance valid-q counts across cores (4 rows each): greedy LPT, then
    # pairwise-swap refinement to minimize the max core sum (which sets the
    # compiled chunk count for every core).
    order = np.argsort(-qlen, kind="stable")
    sums = [0] * NCORES
    counts = [0] * NCORES
    rows_per_core = [[] for _ in range(NCORES)]
    for b in order:
        cands = [c for c in range(NCORES) if counts[c] < BPC]
        c = min(cands, key=lambda c: sums[c])
        rows_per_core[c].append(int(b))
        sums[c] += int(qlen[b])
        counts[c] += 1
    improved = True
    while improved:
        improved = False
        hi = int(np.argmax(sums))
        for lo in sorted(range(NCORES), key=lambda c: sums[c]):
            if lo == hi:
                continue
            for i, bh in enumerate(rows_per_core[hi]):
                for k, bl in enumerate(rows_per_core[lo]):
                    delta = int(qlen[bh]) - int(qlen[bl])
                    if delta <= 0:
                        continue
                    new_hi = sums[hi] - delta
                    new_lo = sums[lo] + delta
                    if max(new_hi, new_lo) < sums[hi]:
                        rows_per_core[hi][i], rows_per_core[lo][k] = bl, bh
                        sums[hi], sums[lo] = new_hi, new_lo
                        improved = True
                        break
                if improved:
                    break
            if improved:
                break
    nj = max(1, (max(sums) + 127) // 128)

    # Pos docs sorted by valid-pair count, then DP-partitioned into regions
    # (even doc counts, width <= 1024) minimizing the predicted per-unit
    # bottleneck-engine time.
    plen = p_mask.sum(axis=1).astype(int)
    pairs = (plen + 1) // 2
    doc_order = np.argsort(pairs, kind="stable")
    sp = [int(max(1, pairs[doc_order[i]])) for i in range(B)]

    # Fixed even regions measured faster than DP-partitioned variants: the
    # ascending-width groups keep the first DMA small and the pipeline
    # uniform.  Per region, choose the fold-chain depth G (max over G groups
    # via G-1 relu stages) to balance the ACT (relu), DVE (reduce) and PE
    # (matmul) totals.
    lmax = [int(max(1, plen[doc_order[(r + 1) * DPR - 1]])) for r in range(NREG)]

    def costs(L, G):
        s = max(1, -(-L // G))
        w = DPR * s
        pe = (2 * G - 1) * w * 0.4167 + (2 * G + 1) * 30
        act = (G - 1) * (0.833 * w + 185)
        dve = 1.0417 * w + 130 * (1 if w <= 512 else 2)
        return w, pe, act, dve

    # Measured: PE and DVE are jointly saturated at G=2; deeper chains shed
    # DVE work but raise PE ((2G-1)/G matmul passes per token) and lose.
    gs = (2, 2, 2, 2)
    if any(-(-L // 2) * DPR > 1024 for L in lmax):
        gs = tuple(
            next(g for g in (2, 3, 4, 8) if -(-L // g) * DPR <= 1024)
            for L in lmax
        )
    groups = [
        (DPR, max(1, -(-lmax[r] // gs[r])), gs[r]) for r in range(NREG)
    ]
    return rows_per_core, nj, doc_order, groups


def _prep_pos(pm, pmask, doc_order, groups):
    """Packed [D, sum_r G_r*w_r] bf16 pos tensor.

    Per region (chain depth G): blocks [grp_1-grp_2 | grp_2-grp_3 | ... |
    grp_{G-1}-grp_G | grp_G], each [w_r, D] transposed.  Each doc's valid
    tokens are distributed over G groups of s slots, padded with duplicates
    of token 0 (duplicates never change a max).
    """
    import ml_dtypes

    blocks = []
    d0 = 0
    for nd, s, G in groups:
        grps = [np.zeros((nd * s, D), np.float32) for _ in range(G)]
        for i, c in enumerate(doc_order[d0 : d0 + nd]):
            tok = pm[c][pmask[c]]  # [L, D] valid tokens
            L = len(tok)
            for t in range(G):
                seg = tok[t * s : (t + 1) * s]
                if len(seg) < s:
                    pad = np.repeat(tok[0:1], s - len(seg), axis=0)
                    seg = np.concatenate([seg, pad], axis=0) if len(seg) else pad
                grps[t][i * s : (i + 1) * s] = seg
        for t in range(G - 1):
            blocks.append((grps[t] - grps[t + 1]).T)
        blocks.append(grps[G - 1].T)
        d0 += nd
    pT = np.ascontiguousarray(np.concatenate(blocks, axis=1)).astype(
        ml_dtypes.bfloat16
    )
    return pT


def _prep_in_maps(query_multi, pos_multi, q_mask, p_mask, plan):
    import ml_dtypes

    rows_per_core, nj, doc_order, groups = plan
    qm = np.ascontiguousarray(np.asarray(query_multi, np.float32))
    pm = np.ascontiguousarray(np.asarray(pos_multi, np.float32))
    qmask = np.asarray(q_mask).astype(bool)
    pmask = np.asarray(p_mask).astype(bool)

    pT = _prep_pos(pm, pmask, doc_order, groups)
    ident = np.eye(128, dtype=ml_dtypes.bfloat16)

    in_maps = []
    qohs = []
    for c in range(NCORES):
        qtok = np.zeros((nj * 128, D), np.float32)
        qoh = np.zeros((nj * 128, BPC), np.float32)
        pos = 0
        for i, b in enumerate(rows_per_core[c]):
            tok = qm[b][qmask[b]]
            n = len(tok)
            qtok[pos : pos + n] = tok
            qoh[pos : pos + n, i] = 1.0
            pos += n
        qT = np.ascontiguousarray(qtok.T).astype(ml_dtypes.bfloat16)
        in_maps.append({"pT": pT, "qT": qT, "identity": ident})
        qohs.append(qoh)  # [nj*128, BPC] host-side sum weights
    return in_maps, qohs


def _host_losses(dense_sim, S_late):
    """Float64 replica of the reference softmax/CE/KL tail."""

    def softmax_and_logp(z):
        m = z.max(axis=1, keepdims=True)
        e = np.exp(z - m)
        den = e.sum(axis=1, keepdims=True)
        return e / den, (z - m) - np.log(den)

    zd = dense_sim / TAU
    zl = S_late / TAU
    dp, logp_d = softmax_and_logp(zd)
    lp, logp_l = softmax_and_logp(zl)
    idx = np.arange(B)
    single = -logp_d[idx, idx].mean()
    multi = -logp_l[idx, idx].mean()
    kl = (dp * np.log((dp + EPS) / (lp + EPS))).sum(axis=1).mean()
    return single, multi, kl


def run(inputs: dict, trace: bool = False):
    """Run the spmd kernel; returns (loss tuple, BassKernelResults)."""
    qmask = np.asarray(inputs["q_mask"]).astype(bool)
    pmask = np.asarray(inputs["p_mask"]).astype(bool)
    plan = _plan(qmask, pmask)
    rows_per_core, nj, doc_order, groups = plan

    nc = _get_nc(nj, groups)
    in_maps, qohs = _prep_in_maps(
        inputs["query_multi"], inputs["pos_multi"], qmask, pmask, plan
    )
    res = run_bass_kernel_spmd(nc, in_maps, core_ids=list(range(NCORES)), trace=trace)

    # Assemble S_raw in original (row, doc) order.  Device output is
    # mx[slot, doc, chunk] (+ j-major slab for the last region); the masked
    # sum over q slots is a tiny host einsum.
    nlast = groups[-1][0]
    S_raw = np.zeros((B, B), np.float64)
    for c in range(NCORES):
        mx = np.asarray(res.results[c]["out"], np.float64)  # [128, B-nlast, nj]
        mxl = np.asarray(res.results[c]["outl"], np.float64)  # [128, nj, nlast]
        m_a = mx.transpose(2, 0, 1).reshape(nj * 128, B - nlast)
        m_b = mxl.transpose(1, 0, 2).reshape(nj * 128, nlast)
        mx2 = np.concatenate([m_a, m_b], axis=1)  # [slot, sorted doc]
        block = qohs[c].T @ mx2  # [BPC, B]
        for i, b in enumerate(rows_per_core[c]):
            S_raw[b, doc_order] = block[i]

    t_i = np.maximum(qmask.sum(axis=1), 1).astype(np.float64)
    S_late = S_raw / t_i[:, None]

    qs = np.asarray(inputs["query_single"], np.float64)
    ps = np.asarray(inputs["pos_single"], np.float64)
    dense_sim = qs @ ps.T

    single, multi, kl = _host_losses(dense_sim, S_late)
    total = single + multi + kl
    out = (np.float32(total), np.float32(single), np.float32(multi), np.float32(kl))
    return out, res


def kernel(query_single, pos_single, query_multi, pos_multi, q_mask, p_mask):
    out, _ = run(
        {
            "query_single": query_single,
            "pos_single": pos_single,
            "query_multi": query_multi,
            "pos_multi": pos_multi,
            "q_mask": q_mask,
            "p_mask": p_mask,
        }
    )
    return out
